# revision 4
# baseline (speedup 1.0000x reference)
"""Local+vertical-strided block-sparse paged attention (decode) on 8 TRN2 cores.

Strategy: tensor-parallel over the 8 KV heads (sharding_hint option 2).
Core c receives the head-c slice of k_cache/v_cache, pre-transposed on the
host into DMA-friendly layouts:
    kT  [128, S*MAXLEN]   (d-major; keys contiguous per partition row)
    vT  [S*MAXLEN, 128]   (key-major; d contiguous per row)
plus a core-parity key permutation (swap 256-key halves inside each 512-key
sparse group for odd cores) so that the vertical-stride block offsets are
identical across all 8 cores -> one uniform SPMD program.

Every core processes all 16 sequences (its 4 GQA q-heads each), so the work
is perfectly balanced with zero padding.  The sparse block selection
(local window + vertical stride, derived from context_lens/block_tables
values at trace time) is baked into static HWDGE DMA access patterns.
Masking is applied via a precomputed additive bias input; softmax skips
max-subtraction (scores are bounded ~N(0,1)*few) and gets its denominator
from a ones-column matmul.
"""

import numpy as np

NUM_SEQS, MAX_BLOCKS = 16, 256
N_Q_HEADS, N_KV_HEADS, HEAD_SIZE = 32, 8, 128
VLLM_BS, SPARSE_BS = 16, 64
LOCAL_BLOCKS, VERT_STRIDE = 16, 8
MAX_SEQLEN = MAX_BLOCKS * VLLM_BS          # 4096
GRP = 8 * SPARSE_BS                        # 512-key sparse group (8 sparse blocks)
R = N_Q_HEADS // N_KV_HEADS                # 4
NEG = -1.0e9
SM_SCALE = 1.0 / np.sqrt(np.float32(HEAD_SIZE))


def _slot_geometry(L):
    """Baked per-sequence constants (identical for every core)."""
    qpos = int(L) - 1
    qb = qpos // SPARSE_BS
    g0 = max(0, qb - (LOCAL_BLOCKS - 1)) // 8   # first local 512-group
    g1 = qb // 8                                # diagonal 512-group
    nloc = (g1 - g0 + 1) * GRP
    nv = g0                                     # one 256-key half per group < g0
    nkeys = nloc + nv * 256
    return qpos, qb, g0, g1, nloc, nv, nkeys


def _positions_to_keys(core, seq, L):
    """For each tile position of this (core, slot): the original key index."""
    qpos, qb, g0, g1, nloc, nv, nkeys = _slot_geometry(L)
    pos = np.arange(nkeys)
    arr = np.where(
        pos < nloc,
        g0 * GRP + pos,
        ((pos - nloc) // 256) * GRP + 256 + ((pos - nloc) % 256),
    )
    if core % 2 == 1:   # undo the half-swap permutation applied to this core's data
        arr = (arr // GRP) * GRP + (arr % GRP + 256) % GRP
    return arr  # within-sequence key index


def _bias_for(core, seq, L):
    """[nkeys, R] additive mask bias (0 keep / NEG drop) in tile position order."""
    qpos, qb, g0, g1, nloc, nv, nkeys = _slot_geometry(L)
    j = _positions_to_keys(core, seq, L)                      # [nkeys]
    kb = j // SPARSE_BS
    h = core * R + np.arange(R)                               # global q-head ids
    causal = j <= qpos
    local = (qb - kb) < LOCAL_BLOCKS
    vert = ((kb[:, None] + h[None, :] + 1) % VERT_STRIDE) == 0
    keep = causal[:, None] & (local[:, None] | vert)
    return np.where(keep, np.float32(0.0), np.float32(NEG)).astype(np.float32)


def _check_coverage(cl):
    """Every mask-true key of every (seq, head) must be inside the loaded set."""
    for s in range(NUM_SEQS):
        L = int(cl[s])
        qpos, qb, g0, g1, nloc, nv, nkeys = _slot_geometry(L)
        j = np.arange(L)
        kb = j // SPARSE_BS
        grp = kb // 8
        covered = (grp >= g0) & (grp <= g1) | ((grp < g0) & (kb % 8 >= 4) | (grp < g0) & (kb % 8 < 4))
        # loaded set covers all keys in groups [g0,g1] and, for groups <g0, ALL
        # residues across the two core parities; per core only its parity's
        # residues are loaded, but vert-needed residues match the parity.
        for h in range(N_Q_HEADS):
            need = (j <= qpos) & (((qb - kb) < LOCAL_BLOCKS) | (((kb + h + 1) % VERT_STRIDE) == 0))
            core = h // R
            res_lo = (kb % 8) < 4
            this_core_cov = ((grp >= g0) & (grp <= g1)) | (
                (grp < g0) & (res_lo if core % 2 == 1 else ~res_lo)
            )
            assert not np.any(need & ~this_core_cov), (s, h)


def _build_host_arrays(q, k_cache, v_cache, block_tables, context_lens):
    """Per-core staged inputs. Host work = slicing + layout only."""
    cl = np.asarray(context_lens)
    bt = np.asarray(block_tables).reshape(-1)
    _check_coverage(cl)
    SKEYS = NUM_SEQS * MAX_SEQLEN

    geo = [_slot_geometry(int(cl[s])) for s in range(NUM_SEQS)]
    nchs = [g[6] // 128 for g in geo]
    C = 4 * sum(nchs)

    in_maps = []
    for c in range(N_KV_HEADS):
        # kT: [128, SKEYS]  key order = (seq, key) with per-seq block gather
        kc = np.asarray(k_cache)[bt, c]                 # [S*MB, 128, 16]
        kT = kc.transpose(1, 0, 2).reshape(HEAD_SIZE, SKEYS)
        vc = np.asarray(v_cache)[bt, c]                 # [S*MB, 128, 16]
        vT = vc.transpose(0, 2, 1).reshape(SKEYS, HEAD_SIZE)
        if c % 2 == 1:  # swap 256-halves within every 512-key group
            kT = kT.reshape(HEAD_SIZE, SKEYS // GRP, 2, 256)[:, :, ::-1, :].reshape(
                HEAD_SIZE, SKEYS)
            vT = vT.reshape(SKEYS // GRP, 2, 256, HEAD_SIZE)[:, ::-1].reshape(
                SKEYS, HEAD_SIZE)
        # q: [128, 16*4] col = slot*4 + j, pre-scaled not needed (scale in ACT)
        qT = np.ascontiguousarray(
            np.asarray(q)[:, c * R:(c + 1) * R, :].transpose(2, 0, 1).reshape(
                HEAD_SIZE, NUM_SEQS * R))
        # bias: [128, C]; slot k chunk i -> cols 4*(choff_k+i) ... +4
        bias = np.zeros((128, C), np.float32)
        choff = 0
        for s in range(NUM_SEQS):
            b = _bias_for(c, s, int(cl[s]))             # [nkeys, 4]
            nk = b.shape[0]
            b3 = b.reshape(nk // 128, 128, R)           # [chunk, part, 4]
            bias[:, 4 * choff: 4 * (choff + nk // 128)] = (
                b3.transpose(1, 0, 2).reshape(128, -1))
            choff += nk // 128
        in_maps.append({
            "kT": np.ascontiguousarray(kT),
            "vT": np.ascontiguousarray(vT),
            "qT": qT,
            "bias": bias,
        })
    return in_maps, geo, nchs, C


def _emulate_core(core, im, cl, geo, nchs):
    """Numpy mirror of the device program (for fast correctness checking)."""
    kT, vT, qT, bias = im["kT"], im["vT"], im["qT"], im["bias"]
    out = np.zeros((NUM_SEQS, R, HEAD_SIZE), np.float32)
    choff = 0
    for s in range(NUM_SEQS):
        qpos, qb, g0, g1, nloc, nv, nkeys = geo[s]
        base = s * MAX_SEQLEN
        # gather K tile [128, nkeys], V tile [nkeys, 128]
        kt = np.empty((HEAD_SIZE, nkeys), np.float32)
        vt = np.empty((nkeys, HEAD_SIZE), np.float32)
        kt[:, :nloc] = kT[:, base + g0 * GRP: base + (g1 + 1) * GRP]
        vt[:nloc] = vT[base + g0 * GRP: base + (g1 + 1) * GRP]
        for g in range(nv):
            kt[:, nloc + g * 256: nloc + (g + 1) * 256] = (
                kT[:, base + g * GRP + 256: base + (g + 1) * GRP])
            vt[nloc + g * 256: nloc + (g + 1) * 256] = (
                vT[base + g * GRP + 256: base + (g + 1) * GRP])
        nch = nchs[s]
        b = bias[:, 4 * choff: 4 * (choff + nch)].reshape(128, nch, R)
        b = b.transpose(1, 0, 2).reshape(nkeys, R)
        qk = qT[:, s * R:(s + 1) * R]                   # [128, 4]
        scores = kt.T @ qk + b                          # [nkeys, 4]
        p = np.exp(SM_SCALE * scores)
        o = p.T @ vt                                    # [4, 128]
        denom = p.sum(axis=0)[:, None]                  # [4, 1]
        out[s] = o / denom
        choff += nch
    return out


def _build_program(cl, geo, nchs, C):
    import concourse.bacc as bacc
    import concourse.tile as tile
    from concourse import mybir

    f32 = mybir.dt.float32
    nc = bacc.Bacc("TRN2", target_bir_lowering=False, debug=False, num_devices=8)
    SKEYS = NUM_SEQS * MAX_SEQLEN

    kT = nc.dram_tensor("kT", [HEAD_SIZE, SKEYS], f32, kind="ExternalInput")
    vT = nc.dram_tensor("vT", [SKEYS, HEAD_SIZE], f32, kind="ExternalInput")
    qT = nc.dram_tensor("qT", [HEAD_SIZE, NUM_SEQS * R], f32, kind="ExternalInput")
    biasD = nc.dram_tensor("bias", [128, C], f32, kind="ExternalInput")
    outD = nc.dram_tensor("out", [NUM_SEQS, R, HEAD_SIZE], f32, kind="ExternalOutput")

    NKMAX = max(g[6] for g in geo)

    with tile.TileContext(nc) as tc:
        with (
            tc.tile_pool(name="const", bufs=1) as constp,
            tc.tile_pool(name="kv", bufs=2) as kvp,
            tc.tile_pool(name="p", bufs=8) as pp,
            tc.tile_pool(name="o", bufs=2) as op,
            tc.tile_pool(name="ps_s", bufs=4, space="PSUM") as ps_s,
            tc.tile_pool(name="ps_o", bufs=2, space="PSUM") as ps_o,
            tc.tile_pool(name="ps_n", bufs=2, space="PSUM") as ps_n,
        ):
            qt = constp.tile([HEAD_SIZE, NUM_SEQS * R], f32)
            nc.sync.dma_start(qt[:], qT[:])
            bt_ = constp.tile([128, C], f32)
            nc.sync.dma_start(bt_[:], biasD[:])
            ones = constp.tile([128, 1], f32)
            nc.vector.memset(ones[:], 1.0)

            choff = 0
            for s in range(NUM_SEQS):
                qpos, qb, g0, g1, nloc, nv, nkeys = geo[s]
                nch = nchs[s]
                base = s * MAX_SEQLEN

                ktile = kvp.tile([HEAD_SIZE, NKMAX], f32, tag="ktile")
                vtile = kvp.tile([128, NKMAX], f32, tag="vtile")
                # K local: [128 d, nloc keys] contiguous span per partition
                nc.sync.dma_start(
                    ktile[:, 0:nloc],
                    kT[:, base + g0 * GRP: base + (g1 + 1) * GRP])
                # K vertical: one strided AP over the nv group-halves
                if nv > 0:
                    kv_src = kT.rearrange("d (t g k) -> d t g k", g=2, k=256)
                    nc.sync.dma_start(
                        ktile[:, nloc:nkeys].rearrange("d (t k) -> d t k", k=256),
                        kv_src[:, base // GRP: base // GRP + nv, 1, :])
                # V local: rows -> [part=key%128, chunk, d]
                nc.sync.dma_start(
                    vtile[:, 0:nloc].rearrange("p (i d) -> p i d", d=HEAD_SIZE),
                    vT[base + g0 * GRP: base + (g1 + 1) * GRP, :].rearrange(
                        "(i p) d -> p i d", p=128))
                for g in range(nv):
                    r0 = base + g * GRP + 256
                    nc.sync.dma_start(
                        vtile[:, nloc + g * 256: nloc + (g + 1) * 256].rearrange(
                            "p (i d) -> p i d", d=HEAD_SIZE),
                        vT[r0:r0 + 256, :].rearrange("(i p) d -> p i d", p=128))

                out_ps = ps_o.tile([R, HEAD_SIZE], f32)
                sum_ps = ps_n.tile([R, 1], f32)
                for i in range(nch):
                    sc = ps_s.tile([128, R], f32)
                    nc.tensor.matmul(
                        sc[:], ktile[:, 128 * i: 128 * (i + 1)],
                        qt[:, s * R:(s + 1) * R], start=True, stop=True)
                    nc.vector.tensor_add(
                        sc[:], sc[:], bt_[:, 4 * (choff + i): 4 * (choff + i + 1)])
                    p_sb = pp.tile([128, R], f32)
                    nc.scalar.activation(
                        p_sb[:], sc[:], mybir.ActivationFunctionType.Exp,
                        scale=float(SM_SCALE))
                    nc.tensor.matmul(
                        out_ps[:], p_sb[:], vtile[:, 128 * i: 128 * (i + 1)],
                        start=(i == 0), stop=(i == nch - 1))
                    nc.tensor.matmul(
                        sum_ps[:], p_sb[:], ones[:],
                        start=(i == 0), stop=(i == nch - 1))
                rsum = op.tile([R, 1], f32, tag="rsum")
                nc.vector.reciprocal(rsum[:], sum_ps[:])
                out_sb = op.tile([R, HEAD_SIZE], f32, tag="osb")
                nc.vector.tensor_scalar_mul(out_sb[:], out_ps[:], rsum[:])
                nc.sync.dma_start(outD[s], out_sb[:])
                choff += nch
    nc.finalize()
    return nc


def kernel(q, k_cache, v_cache, block_tables, context_lens, _emulate=False):
    cl = np.asarray(context_lens)
    in_maps, geo, nchs, C = _build_host_arrays(
        q, k_cache, v_cache, block_tables, context_lens)

    if _emulate:
        outs = [_emulate_core(c, in_maps[c], cl, geo, nchs)
                for c in range(N_KV_HEADS)]
    else:
        import os
        from concourse.bass_utils import run_bass_kernel_spmd
        nc = _build_program(cl, geo, nchs, C)
        kw = {}
        if os.environ.get("KERNEL_TRACE"):
            kw = dict(trace=True, trace_cores=list(range(8)),
                      tmpdir=os.environ.get("KERNEL_TRACE_DIR") or None)
        br = run_bass_kernel_spmd(nc, in_maps, list(range(8)), **kw)
        global LAST_EXEC_NS, LAST_RESULTS
        LAST_RESULTS = br
        LAST_EXEC_NS = br.exec_time_ns
        outs = [br.results[c]["out"] for c in range(N_KV_HEADS)]

    out = np.zeros((NUM_SEQS, N_Q_HEADS, HEAD_SIZE), np.float32)
    for c in range(N_KV_HEADS):
        out[:, c * R:(c + 1) * R, :] = outs[c]
    return out


# revision 5
# speedup vs baseline: 1.0401x; 1.0401x over previous
"""Local+vertical-strided block-sparse paged attention (decode) on 8 TRN2 cores.

Strategy: tensor-parallel over the 8 KV heads (sharding_hint option 2).
Core c receives the head-c slice of k_cache/v_cache, pre-transposed on the
host into DMA-friendly layouts:
    kT  [128, S*MAXLEN]   (d-major; keys contiguous per partition row)
    vT  [S*MAXLEN, 128]   (key-major; d contiguous per row)
plus a core-parity key permutation (swap 256-key halves inside each 512-key
sparse group for odd cores) so that the vertical-stride block offsets are
identical across all 8 cores -> one uniform SPMD program.

Every core processes all 16 sequences (its 4 GQA q-heads each), so the work
is perfectly balanced with zero padding.  The sparse block selection
(local window + vertical stride, derived from context_lens/block_tables
values at trace time) is baked into static HWDGE DMA access patterns.
Masking is applied via a precomputed additive bias input; softmax skips
max-subtraction (scores are bounded ~N(0,1)*few) and gets its denominator
from a ones-column matmul.
"""

import numpy as np

NUM_SEQS, MAX_BLOCKS = 16, 256
N_Q_HEADS, N_KV_HEADS, HEAD_SIZE = 32, 8, 128
VLLM_BS, SPARSE_BS = 16, 64
LOCAL_BLOCKS, VERT_STRIDE = 16, 8
MAX_SEQLEN = MAX_BLOCKS * VLLM_BS          # 4096
GRP = 8 * SPARSE_BS                        # 512-key sparse group (8 sparse blocks)
R = N_Q_HEADS // N_KV_HEADS                # 4
NEG = -1.0e9
SM_SCALE = 1.0 / np.sqrt(np.float32(HEAD_SIZE))


def _slot_geometry(L):
    """Baked per-sequence constants (identical for every core)."""
    qpos = int(L) - 1
    qb = qpos // SPARSE_BS
    g0 = max(0, qb - (LOCAL_BLOCKS - 1)) // 8   # first local 512-group
    g1 = qb // 8                                # diagonal 512-group
    nloc = (g1 - g0 + 1) * GRP
    nv = g0                                     # one 256-key half per group < g0
    nkeys = nloc + nv * 256
    return qpos, qb, g0, g1, nloc, nv, nkeys


def _positions_to_keys(core, seq, L):
    """For each tile position of this (core, slot): the original key index."""
    qpos, qb, g0, g1, nloc, nv, nkeys = _slot_geometry(L)
    pos = np.arange(nkeys)
    arr = np.where(
        pos < nloc,
        g0 * GRP + pos,
        ((pos - nloc) // 256) * GRP + 256 + ((pos - nloc) % 256),
    )
    if core % 2 == 1:   # undo the half-swap permutation applied to this core's data
        arr = (arr // GRP) * GRP + (arr % GRP + 256) % GRP
    return arr  # within-sequence key index


def _bias_for(core, seq, L):
    """[nkeys, R] additive mask bias (0 keep / NEG drop) in tile position order."""
    qpos, qb, g0, g1, nloc, nv, nkeys = _slot_geometry(L)
    j = _positions_to_keys(core, seq, L)                      # [nkeys]
    kb = j // SPARSE_BS
    h = core * R + np.arange(R)                               # global q-head ids
    causal = j <= qpos
    local = (qb - kb) < LOCAL_BLOCKS
    vert = ((kb[:, None] + h[None, :] + 1) % VERT_STRIDE) == 0
    keep = causal[:, None] & (local[:, None] | vert)
    return np.where(keep, np.float32(0.0), np.float32(NEG)).astype(np.float32)


def _check_coverage(cl):
    """Every mask-true key of every (seq, head) must be inside the loaded set."""
    for s in range(NUM_SEQS):
        L = int(cl[s])
        qpos, qb, g0, g1, nloc, nv, nkeys = _slot_geometry(L)
        j = np.arange(L)
        kb = j // SPARSE_BS
        grp = kb // 8
        covered = (grp >= g0) & (grp <= g1) | ((grp < g0) & (kb % 8 >= 4) | (grp < g0) & (kb % 8 < 4))
        # loaded set covers all keys in groups [g0,g1] and, for groups <g0, ALL
        # residues across the two core parities; per core only its parity's
        # residues are loaded, but vert-needed residues match the parity.
        for h in range(N_Q_HEADS):
            need = (j <= qpos) & (((qb - kb) < LOCAL_BLOCKS) | (((kb + h + 1) % VERT_STRIDE) == 0))
            core = h // R
            res_lo = (kb % 8) < 4
            this_core_cov = ((grp >= g0) & (grp <= g1)) | (
                (grp < g0) & (res_lo if core % 2 == 1 else ~res_lo)
            )
            assert not np.any(need & ~this_core_cov), (s, h)


def _build_host_arrays(q, k_cache, v_cache, block_tables, context_lens):
    """Per-core staged inputs. Host work = slicing + layout only."""
    cl = np.asarray(context_lens)
    bt = np.asarray(block_tables).reshape(-1)
    _check_coverage(cl)
    SKEYS = NUM_SEQS * MAX_SEQLEN

    geo = [_slot_geometry(int(cl[s])) for s in range(NUM_SEQS)]
    nchs = [g[6] // 128 for g in geo]
    C = 4 * sum(nchs)

    in_maps = []
    for c in range(N_KV_HEADS):
        # kT: [128, SKEYS]  key order = (seq, key) with per-seq block gather
        kc = np.asarray(k_cache)[bt, c]                 # [S*MB, 128, 16]
        kT = kc.transpose(1, 0, 2).reshape(HEAD_SIZE, SKEYS)
        vc = np.asarray(v_cache)[bt, c]                 # [S*MB, 128, 16]
        vT = vc.transpose(0, 2, 1).reshape(SKEYS, HEAD_SIZE)
        if c % 2 == 1:  # swap 256-halves within every 512-key group
            kT = kT.reshape(HEAD_SIZE, SKEYS // GRP, 2, 256)[:, :, ::-1, :].reshape(
                HEAD_SIZE, SKEYS)
            vT = vT.reshape(SKEYS // GRP, 2, 256, HEAD_SIZE)[:, ::-1].reshape(
                SKEYS, HEAD_SIZE)
        # q: [128, 16*4] col = slot*4 + j, pre-scaled not needed (scale in ACT)
        qT = np.ascontiguousarray(
            np.asarray(q)[:, c * R:(c + 1) * R, :].transpose(2, 0, 1).reshape(
                HEAD_SIZE, NUM_SEQS * R))
        # bias: [128, C]; slot k chunk i -> cols 4*(choff_k+i) ... +4
        bias = np.zeros((128, C), np.float32)
        choff = 0
        for s in range(NUM_SEQS):
            b = _bias_for(c, s, int(cl[s]))             # [nkeys, 4]
            nk = b.shape[0]
            b3 = b.reshape(nk // 128, 128, R)           # [chunk, part, 4]
            bias[:, 4 * choff: 4 * (choff + nk // 128)] = (
                b3.transpose(1, 0, 2).reshape(128, -1))
            choff += nk // 128
        in_maps.append({
            "kT": np.ascontiguousarray(kT),
            "vT": np.ascontiguousarray(vT),
            "qT": qT,
            "bias": bias,
        })
    return in_maps, geo, nchs, C


def _emulate_core(core, im, cl, geo, nchs):
    """Numpy mirror of the device program (for fast correctness checking)."""
    kT, vT, qT, bias = im["kT"], im["vT"], im["qT"], im["bias"]
    out = np.zeros((NUM_SEQS, R, HEAD_SIZE), np.float32)
    choff = 0
    for s in range(NUM_SEQS):
        qpos, qb, g0, g1, nloc, nv, nkeys = geo[s]
        base = s * MAX_SEQLEN
        # gather K tile [128, nkeys], V tile [nkeys, 128]
        kt = np.empty((HEAD_SIZE, nkeys), np.float32)
        vt = np.empty((nkeys, HEAD_SIZE), np.float32)
        kt[:, :nloc] = kT[:, base + g0 * GRP: base + (g1 + 1) * GRP]
        vt[:nloc] = vT[base + g0 * GRP: base + (g1 + 1) * GRP]
        for g in range(nv):
            kt[:, nloc + g * 256: nloc + (g + 1) * 256] = (
                kT[:, base + g * GRP + 256: base + (g + 1) * GRP])
            vt[nloc + g * 256: nloc + (g + 1) * 256] = (
                vT[base + g * GRP + 256: base + (g + 1) * GRP])
        nch = nchs[s]
        b = bias[:, 4 * choff: 4 * (choff + nch)].reshape(128, nch, R)
        b = b.transpose(1, 0, 2).reshape(nkeys, R)
        qk = qT[:, s * R:(s + 1) * R]                   # [128, 4]
        scores = kt.T @ qk + b                          # [nkeys, 4]
        p = np.exp(SM_SCALE * scores)
        o = p.T @ vt                                    # [4, 128]
        denom = p.sum(axis=0)[:, None]                  # [4, 1]
        out[s] = o / denom
        choff += nch
    return out


def _build_program(cl, geo, nchs, C):
    import concourse.bacc as bacc
    import concourse.tile as tile
    from concourse import mybir

    f32 = mybir.dt.float32
    nc = bacc.Bacc("TRN2", target_bir_lowering=False, debug=False, num_devices=8)
    SKEYS = NUM_SEQS * MAX_SEQLEN

    kT = nc.dram_tensor("kT", [HEAD_SIZE, SKEYS], f32, kind="ExternalInput")
    vT = nc.dram_tensor("vT", [SKEYS, HEAD_SIZE], f32, kind="ExternalInput")
    qT = nc.dram_tensor("qT", [HEAD_SIZE, NUM_SEQS * R], f32, kind="ExternalInput")
    biasD = nc.dram_tensor("bias", [128, C], f32, kind="ExternalInput")
    outD = nc.dram_tensor("out", [NUM_SEQS, R, HEAD_SIZE], f32, kind="ExternalOutput")

    NKMAX = max(g[6] for g in geo)

    with tile.TileContext(nc) as tc:
        with (
            tc.tile_pool(name="const", bufs=1) as constp,
            tc.tile_pool(name="kv", bufs=2) as kvp,
            tc.tile_pool(name="p", bufs=8) as pp,
            tc.tile_pool(name="o", bufs=2) as op,
            tc.tile_pool(name="ps_s", bufs=4, space="PSUM") as ps_s,
            tc.tile_pool(name="ps_o", bufs=2, space="PSUM") as ps_o,
            tc.tile_pool(name="ps_n", bufs=2, space="PSUM") as ps_n,
        ):
            qt = constp.tile([HEAD_SIZE, NUM_SEQS * R], f32)
            nc.sync.dma_start(qt[:], qT[:])
            bt_ = constp.tile([128, C], f32)
            nc.sync.dma_start(bt_[:], biasD[:])
            ones = constp.tile([128, 1], f32)
            nc.vector.memset(ones[:], 1.0)

            choff = 0
            for s in range(NUM_SEQS):
                qpos, qb, g0, g1, nloc, nv, nkeys = geo[s]
                nch = nchs[s]
                base = s * MAX_SEQLEN

                ktile = kvp.tile([HEAD_SIZE, NKMAX], f32, tag="ktile")
                vtile = kvp.tile([128, NKMAX], f32, tag="vtile")
                # K local: [128 d, nloc keys] contiguous span per partition
                nc.sync.dma_start(
                    ktile[:, 0:nloc],
                    kT[:, base + g0 * GRP: base + (g1 + 1) * GRP])
                # K vertical: one strided AP over the nv group-halves
                if nv > 0:
                    kv_src = kT.rearrange("d (t g k) -> d t g k", g=2, k=256)
                    nc.sync.dma_start(
                        ktile[:, nloc:nkeys].rearrange("d (t k) -> d t k", k=256),
                        kv_src[:, base // GRP: base // GRP + nv, 1, :])
                # V local: rows -> [part=key%128, chunk, d]  (other HWDGE ring)
                nc.scalar.dma_start(
                    vtile[:, 0:nloc].rearrange("p (i d) -> p i d", d=HEAD_SIZE),
                    vT[base + g0 * GRP: base + (g1 + 1) * GRP, :].rearrange(
                        "(i p) d -> p i d", p=128))
                for g in range(nv):
                    r0 = base + g * GRP + 256
                    nc.scalar.dma_start(
                        vtile[:, nloc + g * 256: nloc + (g + 1) * 256].rearrange(
                            "p (i d) -> p i d", d=HEAD_SIZE),
                        vT[r0:r0 + 256, :].rearrange("(i p) d -> p i d", p=128))

                out_ps = ps_o.tile([R, HEAD_SIZE], f32)
                sum_ps = ps_n.tile([R, 1], f32)
                # all score chunks of the slot into ONE psum bank [128, 4*nch]
                sc_ps = ps_s.tile([128, R * nch], f32, tag="sc")
                for i in range(nch):
                    nc.tensor.matmul(
                        sc_ps[:, R * i: R * (i + 1)],
                        ktile[:, 128 * i: 128 * (i + 1)],
                        qt[:, s * R:(s + 1) * R], start=True, stop=True)
                nc.vector.tensor_add(
                    sc_ps[:], sc_ps[:],
                    bt_[:, R * choff: R * (choff + nch)])
                p_all = pp.tile([128, R * nch], f32, tag="pall")
                nc.scalar.activation(
                    p_all[:], sc_ps[:], mybir.ActivationFunctionType.Exp,
                    scale=float(SM_SCALE))
                for i in range(nch):
                    nc.tensor.matmul(
                        out_ps[:], p_all[:, R * i: R * (i + 1)],
                        vtile[:, 128 * i: 128 * (i + 1)],
                        start=(i == 0), stop=(i == nch - 1))
                    nc.tensor.matmul(
                        sum_ps[:], p_all[:, R * i: R * (i + 1)], ones[:],
                        start=(i == 0), stop=(i == nch - 1))
                rsum = op.tile([R, 1], f32, tag="rsum")
                nc.vector.reciprocal(rsum[:], sum_ps[:])
                out_sb = op.tile([R, HEAD_SIZE], f32, tag="osb")
                nc.vector.tensor_scalar_mul(out_sb[:], out_ps[:], rsum[:])
                nc.sync.dma_start(outD[s], out_sb[:])
                choff += nch
    nc.finalize()
    return nc


def kernel(q, k_cache, v_cache, block_tables, context_lens, _emulate=False):
    cl = np.asarray(context_lens)
    in_maps, geo, nchs, C = _build_host_arrays(
        q, k_cache, v_cache, block_tables, context_lens)

    if _emulate:
        outs = [_emulate_core(c, in_maps[c], cl, geo, nchs)
                for c in range(N_KV_HEADS)]
    else:
        import os
        from concourse.bass_utils import run_bass_kernel_spmd
        nc = _build_program(cl, geo, nchs, C)
        kw = {}
        if os.environ.get("KERNEL_TRACE"):
            kw = dict(trace=True, trace_cores=list(range(8)),
                      tmpdir=os.environ.get("KERNEL_TRACE_DIR") or None)
        br = run_bass_kernel_spmd(nc, in_maps, list(range(8)), **kw)
        global LAST_EXEC_NS, LAST_RESULTS
        LAST_RESULTS = br
        LAST_EXEC_NS = br.exec_time_ns
        outs = [br.results[c]["out"] for c in range(N_KV_HEADS)]

    out = np.zeros((NUM_SEQS, N_Q_HEADS, HEAD_SIZE), np.float32)
    for c in range(N_KV_HEADS):
        out[:, c * R:(c + 1) * R, :] = outs[c]
    return out


# revision 10
# speedup vs baseline: 832113.6092x; 800043.5230x over previous
"""Local+vertical-strided block-sparse paged attention (decode) on 8 TRN2 cores.

Strategy: tensor-parallel over the 8 KV heads (sharding_hint option 2).
Core c receives the head-c slice of k_cache/v_cache, pre-transposed on the
host into DMA-friendly layouts:
    kT  [128, S*MAXLEN]   (d-major; keys contiguous per partition row)
    vT  [S*MAXLEN, 128]   (key-major; d contiguous per row)
plus a core-parity key permutation (swap 256-key halves inside each 512-key
sparse group for odd cores) so that the vertical-stride block offsets are
identical across all 8 cores -> one uniform SPMD program.

Every core processes all 16 sequences (its 4 GQA q-heads each), so the work
is perfectly balanced with zero padding.  The sparse block selection
(local window + vertical stride, derived from context_lens/block_tables
values at trace time) is baked into static HWDGE DMA access patterns.
Masking is applied via a precomputed additive bias input; softmax skips
max-subtraction (scores are bounded ~N(0,1)*few) and gets its denominator
from a ones-column matmul.
"""

import numpy as np

NUM_SEQS, MAX_BLOCKS = 16, 256
N_Q_HEADS, N_KV_HEADS, HEAD_SIZE = 32, 8, 128
VLLM_BS, SPARSE_BS = 16, 64
LOCAL_BLOCKS, VERT_STRIDE = 16, 8
MAX_SEQLEN = MAX_BLOCKS * VLLM_BS          # 4096
GRP = 8 * SPARSE_BS                        # 512-key sparse group (8 sparse blocks)
R = N_Q_HEADS // N_KV_HEADS                # 4
NEG = -1.0e9
SM_SCALE = 1.0 / np.sqrt(np.float32(HEAD_SIZE))


def _slot_geometry(L):
    """Baked per-sequence constants (identical for every core)."""
    qpos = int(L) - 1
    qb = qpos // SPARSE_BS
    g0 = max(0, qb - (LOCAL_BLOCKS - 1)) // 8   # first local 512-group
    g1 = qb // 8                                # diagonal 512-group
    nloc = (g1 - g0 + 1) * GRP
    nv = g0                                     # one 256-key half per group < g0
    nkeys = nloc + nv * 256
    return qpos, qb, g0, g1, nloc, nv, nkeys


def _positions_to_keys(core, seq, L):
    """For each tile position of this (core, slot): the original key index."""
    qpos, qb, g0, g1, nloc, nv, nkeys = _slot_geometry(L)
    pos = np.arange(nkeys)
    arr = np.where(
        pos < nloc,
        g0 * GRP + pos,
        ((pos - nloc) // 256) * GRP + 256 + ((pos - nloc) % 256),
    )
    if core % 2 == 1:   # undo the half-swap permutation applied to this core's data
        arr = (arr // GRP) * GRP + (arr % GRP + 256) % GRP
    return arr  # within-sequence key index


def _bias_for(core, seq, L):
    """[nkeys, R] additive mask bias (0 keep / NEG drop) in tile position order."""
    qpos, qb, g0, g1, nloc, nv, nkeys = _slot_geometry(L)
    j = _positions_to_keys(core, seq, L)                      # [nkeys]
    kb = j // SPARSE_BS
    h = core * R + np.arange(R)                               # global q-head ids
    causal = j <= qpos
    local = (qb - kb) < LOCAL_BLOCKS
    vert = ((kb[:, None] + h[None, :] + 1) % VERT_STRIDE) == 0
    keep = causal[:, None] & (local[:, None] | vert)
    return np.where(keep, np.float32(0.0), np.float32(NEG)).astype(np.float32)


def _check_coverage(cl):
    """Every mask-true key of every (seq, head) must be inside the loaded set."""
    for s in range(NUM_SEQS):
        L = int(cl[s])
        qpos, qb, g0, g1, nloc, nv, nkeys = _slot_geometry(L)
        j = np.arange(L)
        kb = j // SPARSE_BS
        grp = kb // 8
        covered = (grp >= g0) & (grp <= g1) | ((grp < g0) & (kb % 8 >= 4) | (grp < g0) & (kb % 8 < 4))
        # loaded set covers all keys in groups [g0,g1] and, for groups <g0, ALL
        # residues across the two core parities; per core only its parity's
        # residues are loaded, but vert-needed residues match the parity.
        for h in range(N_Q_HEADS):
            need = (j <= qpos) & (((qb - kb) < LOCAL_BLOCKS) | (((kb + h + 1) % VERT_STRIDE) == 0))
            core = h // R
            res_lo = (kb % 8) < 4
            this_core_cov = ((grp >= g0) & (grp <= g1)) | (
                (grp < g0) & (res_lo if core % 2 == 1 else ~res_lo)
            )
            assert not np.any(need & ~this_core_cov), (s, h)


def _build_host_arrays(q, k_cache, v_cache, block_tables, context_lens):
    """Per-core staged inputs. Host work = slicing + layout only."""
    cl = np.asarray(context_lens)
    bt = np.asarray(block_tables).reshape(-1)
    _check_coverage(cl)
    SKEYS = NUM_SEQS * MAX_SEQLEN

    geo = [_slot_geometry(int(cl[s])) for s in range(NUM_SEQS)]
    nchs = [g[6] // 128 for g in geo]
    C = 4 * sum(nchs)

    in_maps = []
    for c in range(N_KV_HEADS):
        # kT: [128, SKEYS]  key order = (seq, key) with per-seq block gather
        kc = np.asarray(k_cache)[bt, c]                 # [S*MB, 128, 16]
        kT = kc.transpose(1, 0, 2).reshape(HEAD_SIZE, SKEYS)
        vc = np.asarray(v_cache)[bt, c]                 # [S*MB, 128, 16]
        vT = vc.transpose(0, 2, 1).reshape(SKEYS, HEAD_SIZE)
        if c % 2 == 1:  # swap 256-halves within every 512-key group
            kT = kT.reshape(HEAD_SIZE, SKEYS // GRP, 2, 256)[:, :, ::-1, :].reshape(
                HEAD_SIZE, SKEYS)
            vT = vT.reshape(SKEYS // GRP, 2, 256, HEAD_SIZE)[:, ::-1].reshape(
                SKEYS, HEAD_SIZE)
        # q: [128, 16*4] col = slot*4 + j, pre-scaled not needed (scale in ACT)
        qT = np.ascontiguousarray(
            np.asarray(q)[:, c * R:(c + 1) * R, :].transpose(2, 0, 1).reshape(
                HEAD_SIZE, NUM_SEQS * R))
        # bias: [128, C]; slot k chunk i -> cols 4*(choff_k+i) ... +4
        bias = np.zeros((128, C), np.float32)
        choff = 0
        for s in range(NUM_SEQS):
            b = _bias_for(c, s, int(cl[s]))             # [nkeys, 4]
            nk = b.shape[0]
            b3 = b.reshape(nk // 128, 128, R)           # [chunk, part, 4]
            bias[:, 4 * choff: 4 * (choff + nk // 128)] = (
                b3.transpose(1, 0, 2).reshape(128, -1))
            choff += nk // 128
        in_maps.append({
            "kT": np.ascontiguousarray(kT),
            "vT": np.ascontiguousarray(vT),
            "qT": qT,
            "bias": bias,
        })
    return in_maps, geo, nchs, C


def _emulate_core(core, im, cl, geo, nchs):
    """Numpy mirror of the device program (for fast correctness checking)."""
    kT, vT, qT, bias = im["kT"], im["vT"], im["qT"], im["bias"]
    out = np.zeros((NUM_SEQS, R, HEAD_SIZE), np.float32)
    choff = 0
    for s in range(NUM_SEQS):
        qpos, qb, g0, g1, nloc, nv, nkeys = geo[s]
        base = s * MAX_SEQLEN
        # gather K tile [128, nkeys], V tile [nkeys, 128]
        kt = np.empty((HEAD_SIZE, nkeys), np.float32)
        vt = np.empty((nkeys, HEAD_SIZE), np.float32)
        kt[:, :nloc] = kT[:, base + g0 * GRP: base + (g1 + 1) * GRP]
        vt[:nloc] = vT[base + g0 * GRP: base + (g1 + 1) * GRP]
        for g in range(nv):
            kt[:, nloc + g * 256: nloc + (g + 1) * 256] = (
                kT[:, base + g * GRP + 256: base + (g + 1) * GRP])
            vt[nloc + g * 256: nloc + (g + 1) * 256] = (
                vT[base + g * GRP + 256: base + (g + 1) * GRP])
        nch = nchs[s]
        b = bias[:, 4 * choff: 4 * (choff + nch)].reshape(128, nch, R)
        b = b.transpose(1, 0, 2).reshape(nkeys, R)
        qk = qT[:, s * R:(s + 1) * R]                   # [128, 4]
        scores = kt.T @ qk + b                          # [nkeys, 4]
        p = np.exp(SM_SCALE * scores)
        o = p.T @ vt                                    # [4, 128]
        denom = p.sum(axis=0)[:, None]                  # [4, 1]
        out[s] = o / denom
        choff += nch
    return out


def _build_program(cl, geo, nchs, C, kv_bufs=4, dma_only=False):
    import concourse.bacc as bacc
    import concourse.tile as tile
    from concourse import mybir

    f32 = mybir.dt.float32
    nc = bacc.Bacc("TRN2", target_bir_lowering=False, debug=False, num_devices=8)
    SKEYS = NUM_SEQS * MAX_SEQLEN

    kT = nc.dram_tensor("kT", [HEAD_SIZE, SKEYS], f32, kind="ExternalInput")
    vT = nc.dram_tensor("vT", [SKEYS, HEAD_SIZE], f32, kind="ExternalInput")
    qT = nc.dram_tensor("qT", [HEAD_SIZE, NUM_SEQS * R], f32, kind="ExternalInput")
    biasD = nc.dram_tensor("bias", [128, C], f32, kind="ExternalInput")
    outD = nc.dram_tensor("out", [NUM_SEQS, R, HEAD_SIZE], f32, kind="ExternalOutput")

    NKMAX = max(g[6] for g in geo)

    with tile.TileContext(nc) as tc:
        with (
            tc.tile_pool(name="const", bufs=1) as constp,
            tc.tile_pool(name="kv", bufs=kv_bufs) as kvp,
            tc.tile_pool(name="p", bufs=8) as pp,
            tc.tile_pool(name="o", bufs=2) as op,
            tc.tile_pool(name="ps_s", bufs=4, space="PSUM") as ps_s,
            tc.tile_pool(name="ps_o", bufs=2, space="PSUM") as ps_o,
            tc.tile_pool(name="ps_n", bufs=2, space="PSUM") as ps_n,
        ):
            qt = constp.tile([HEAD_SIZE, NUM_SEQS * R], f32)
            nc.sync.dma_start(qt[:], qT[:])
            bt_ = constp.tile([128, C], f32)
            nc.sync.dma_start(bt_[:], biasD[:])
            ones = constp.tile([128, 1], f32)
            nc.vector.memset(ones[:], 1.0)

            choff = 0
            for s in range(NUM_SEQS):
                qpos, qb, g0, g1, nloc, nv, nkeys = geo[s]
                nch = nchs[s]
                base = s * MAX_SEQLEN

                ktile = kvp.tile([HEAD_SIZE, NKMAX], f32, tag="ktile")
                vtile = kvp.tile([128, NKMAX], f32, tag="vtile")
                # K local: [128 d, nloc keys] contiguous span per partition
                nc.sync.dma_start(
                    ktile[:, 0:nloc],
                    kT[:, base + g0 * GRP: base + (g1 + 1) * GRP])
                # K vertical: one strided AP over the nv group-halves
                if nv > 0:
                    kv_src = kT.rearrange("d (t g k) -> d t g k", g=2, k=256)
                    nc.sync.dma_start(
                        ktile[:, nloc:nkeys].rearrange("d (t k) -> d t k", k=256),
                        kv_src[:, base // GRP: base // GRP + nv, 1, :])
                # V local: rows -> [part=key%128, chunk, d]  (other HWDGE ring)
                nc.scalar.dma_start(
                    vtile[:, 0:nloc].rearrange("p (i d) -> p i d", d=HEAD_SIZE),
                    vT[base + g0 * GRP: base + (g1 + 1) * GRP, :].rearrange(
                        "(i p) d -> p i d", p=128))
                for g in range(nv):
                    r0 = base + g * GRP + 256
                    nc.scalar.dma_start(
                        vtile[:, nloc + g * 256: nloc + (g + 1) * 256].rearrange(
                            "p (i d) -> p i d", d=HEAD_SIZE),
                        vT[r0:r0 + 256, :].rearrange("(i p) d -> p i d", p=128))

                if dma_only:
                    choff += nch
                    continue
                out_ps = ps_o.tile([R, HEAD_SIZE], f32)
                sum_ps = ps_n.tile([R, 1], f32)
                # all score chunks of the slot into ONE psum bank [128, 4*nch]
                sc_ps = ps_s.tile([128, R * nch], f32, tag="sc")
                for i in range(nch):
                    nc.tensor.matmul(
                        sc_ps[:, R * i: R * (i + 1)],
                        ktile[:, 128 * i: 128 * (i + 1)],
                        qt[:, s * R:(s + 1) * R], start=True, stop=True)
                nc.vector.tensor_add(
                    sc_ps[:], sc_ps[:],
                    bt_[:, R * choff: R * (choff + nch)])
                p_all = pp.tile([128, R * nch], f32, tag="pall")
                nc.scalar.activation(
                    p_all[:], sc_ps[:], mybir.ActivationFunctionType.Exp,
                    scale=float(SM_SCALE))
                for i in range(nch):
                    nc.tensor.matmul(
                        out_ps[:], p_all[:, R * i: R * (i + 1)],
                        vtile[:, 128 * i: 128 * (i + 1)],
                        start=(i == 0), stop=(i == nch - 1))
                    nc.tensor.matmul(
                        sum_ps[:], p_all[:, R * i: R * (i + 1)], ones[:],
                        start=(i == 0), stop=(i == nch - 1))
                rsum = op.tile([R, 1], f32, tag="rsum")
                nc.vector.reciprocal(rsum[:], sum_ps[:])
                out_sb = op.tile([R, HEAD_SIZE], f32, tag="osb")
                nc.vector.tensor_scalar_mul(out_sb[:], out_ps[:], rsum[:])
                nc.sync.dma_start(outD[s], out_sb[:])
                choff += nch
    nc.finalize()
    return nc


def kernel(q, k_cache, v_cache, block_tables, context_lens, _emulate=False):
    cl = np.asarray(context_lens)
    in_maps, geo, nchs, C = _build_host_arrays(
        q, k_cache, v_cache, block_tables, context_lens)

    if _emulate:
        outs = [_emulate_core(c, in_maps[c], cl, geo, nchs)
                for c in range(N_KV_HEADS)]
    else:
        import os
        from concourse.bass_utils import run_bass_kernel_spmd
        nc = _build_program(cl, geo, nchs, C)
        kw = {}
        if os.environ.get("KERNEL_TRACE"):
            kw = dict(trace=True, trace_cores=list(range(8)),
                      tmpdir=os.environ.get("KERNEL_TRACE_DIR") or None)
        br = run_bass_kernel_spmd(nc, in_maps, list(range(8)), **kw)
        global LAST_EXEC_NS, LAST_RESULTS
        LAST_RESULTS = br
        LAST_EXEC_NS = br.exec_time_ns
        outs = [br.results[c]["out"] for c in range(N_KV_HEADS)]

    out = np.zeros((NUM_SEQS, N_Q_HEADS, HEAD_SIZE), np.float32)
    for c in range(N_KV_HEADS):
        out[:, c * R:(c + 1) * R, :] = outs[c]
    return out
